# revision 39
# baseline (speedup 1.0000x reference)
"""Trainium2 Bass kernel for nn_DiffusionPolicyHead (EDM/DDIM sampler head).

Strategy (v2)
-------------
Pure data parallel over 8 NeuronCores (batch 32768 -> 4096/core).

Host-side algebra: with a_t = ratio + (1-ratio) c_skip, b_t = (1-ratio) c_out,
substituting action_t = g_t z_t + beta_t gives
    z_{t+1} = z_t + s_t (h3 @ Wout),  s_t = b_t/g_{t+1},  z_0 = init_noise,
and layer 0 is  h0 = relu(alpha_t (z @ W0a) + u + e'_t)  with alpha_t =
c_in g_t a per-step SCALAR, u = state @ W0s step-invariant (computed on the
HOST in f32, stored bf16), and e'_t a per-step bias riding the ACT/DVE bias
slot.

Device layout (per core): feature-major, batch 4096 = 2 halves x 4 blocks of
512. z lives STACKED: zs_H [128, 512] f32 where partition strip 32q..32q+32
holds action dims of block b=4H+q. Per step (88 matmuls, all issuing at the
empirical PE floor of ~216ns/512 cols, dtype-independent):
  - L0: per (H, j, block-pair): a [128, 1024] PSUM tile <- 2 identity-matmuls
    injecting u (bf16, host-precomputed — zero matmul quantization error) +
    2 fp8 matmuls of the stacked z8 against row-strip zero-padded W0a
    variants. alpha_t is folded into the z8 quantization, NOT the weights,
    so fp8 never underflows. The next step's first two L0 tiles are
    pre-started (u-half only), interleaved as prestart/out(H0)/prestart/
    out(H1): they depend only on a free PSUM buffer, filling the PE gap in
    BOTH po wait windows (po_H1 reuses po_H0's buffer, freed by STT_H0).
  - Hidden: fp8 e4m3 DoubleRow, K=256 per matmul, 2 MMs per tile.
  - Out: per H: 4 fp8-DR matmuls with column-strip zero-padded Wout variants
    accumulate the STACKED po [128, 512]; one DVE scalar_tensor_tensor does
    zs' = s_t po + zs; z8 = q8(alpha_{t+1} zs') requantizes on DVE for H0
    (queue-sequential after the STT, no cross-engine hop — it gates the next
    step's first z-matmuls) and on ACT for H1 (engine balance; H1's z-matmuls
    are covered by 4 L0 tiles of slack). No DMA in the z loop.
Epilogues are [128, 1024] relu+bias ops on 2-bank PSUM tiles, 4 in flight —
depth-4 pipelining is what hides epilogue latency (4-bank tiles cap the
pipeline at depth 2 and stall the PE). 17/15 ACT/DVE split with each
(layer, half, block-pair) j-pair on opposite engines and j-pairs spread 4
emission slots apart (pair-adjacent ordering measured slower — emission
slack beats handoff latency). hpool bufs=8 matches the 8 live h tiles.
Per-(step,layer) calibration biases (batch-mean fp8 error from a 512-sample
host run of the quantized pipeline) cancel the batch-coherent quantization
error; a final host-side bias vector bfix absorbs the residual mean.
"""

import os
import sys

sys.path.insert(0, "/opt/trn_rl_repo")

import numpy as np
import ml_dtypes

BATCH, STATE_DIM, ACTION_DIM = 32768, 128, 32
HIDDEN, EMBED, N_STEPS = 256, 64, 50
SIGMA_MAX, SIGMA_MIN, RHO = 80.0, 0.001, 7.0
N_CORES = 8
B_CORE = BATCH // N_CORES  # 4096
NB = 512
CAL_SAMPLES = 512

_cached = {}


def _q8(x):
    return np.asarray(x, ml_dtypes.float8_e4m3).astype(np.float32)


def _bf16(x):
    return np.asarray(x, ml_dtypes.bfloat16).astype(np.float32)


def _host_tables(W0, b0, bout):
    """Per-step diffusion constants (float64)."""
    W0 = W0.astype(np.float64)
    b0 = b0.astype(np.float64)
    bout = bout.astype(np.float64)
    W0a = W0[:ACTION_DIM]
    W0e = W0[ACTION_DIM : ACTION_DIM + EMBED]
    W0s = W0[ACTION_DIM + EMBED :]

    ramp = np.linspace(0.0, 1.0, N_STEPS)
    min_r, max_r = SIGMA_MIN ** (1.0 / RHO), SIGMA_MAX ** (1.0 / RHO)
    sig = np.concatenate([(max_r + ramp * (min_r - max_r)) ** RHO, np.zeros(1)])

    half = EMBED // 2
    freqs = np.exp(-np.log(10000.0) * np.arange(half, dtype=np.float64) / half)

    sd = 1.0
    g = sig[0]
    beta = np.zeros(ACTION_DIM)
    alpha = np.empty(N_STEPS)
    eprime = np.empty((N_STEPS, HIDDEN))
    s_t = np.empty(N_STEPS)
    for t in range(N_STEPS):
        s, sn = sig[t], sig[t + 1]
        var = s * s + sd * sd
        c_in = 1.0 / np.sqrt(var)
        c_skip = sd * sd / var
        c_out = s * sd / np.sqrt(var)
        ratio = sn / s
        a_t = ratio + (1.0 - ratio) * c_skip
        b_t = (1.0 - ratio) * c_out
        ang = np.log(s) * freqs
        emb = np.concatenate([np.sin(ang), np.cos(ang)])
        alpha[t] = c_in * g
        eprime[t] = emb @ W0e + b0 + c_in * (beta @ W0a)
        g_next = a_t * g
        beta = a_t * beta + b_t * bout
        s_t[t] = b_t / g_next
        g = g_next
    return dict(
        alpha=alpha,
        eprime=eprime,
        s_t=s_t.astype(np.float32),
        W0s=W0s.astype(np.float32),
        W0a=W0a.astype(np.float32),
        g_final=g,
        beta_final=beta,
    )


def _calibrate(state, init_noise, Wh, bh, Wout, tb, n_steps):
    """Emulate the v2 device pipeline on a sample; return per-(step,layer)
    bias corrections (db0 for L0, db for hidden) and the final bias bfix."""
    rng = np.random.default_rng(12345)
    idx = rng.choice(BATCH, CAL_SAMPLES, replace=False)
    st = state[idx].astype(np.float32)
    z0 = init_noise[idx].astype(np.float32)
    W0s, W0a = tb["W0s"], tb["W0a"]
    W0a8 = _q8(W0a)
    Wh8 = _q8(Wh)
    Wout8 = _q8(Wout)
    u_ex = st @ W0s
    u_dev = _bf16(u_ex)
    alpha = tb["alpha"].astype(np.float32)
    s_t = tb["s_t"]
    eprime = tb["eprime"].astype(np.float32)

    db0 = np.zeros((n_steps, HIDDEN), np.float32)
    db = np.zeros((n_steps, 3, HIDDEN), np.float32)
    z = z0.copy()
    z_ex = z0.copy()
    for t in range(n_steps):
        pre = _q8(alpha[t] * z) @ W0a8 + u_dev + eprime[t]
        pre_ex = alpha[t] * (z @ W0a) + u_ex + eprime[t]
        db0[t] = -(pre - pre_ex).mean(axis=0)
        h = np.maximum(pre + db0[t], 0.0)
        for l in range(3):
            A = _q8(h) @ Wh8[l]
            db[t, l] = -(A - h @ Wh[l]).mean(axis=0)
            h = np.maximum(A + bh[l] + db[t, l], 0.0)
        z = z + s_t[t] * (_q8(h) @ Wout8)
        # exact path for bfix
        hx = np.maximum(alpha[t] * (z_ex @ W0a) + u_ex + eprime[t], 0.0)
        for l in range(3):
            hx = np.maximum(hx @ Wh[l] + bh[l], 0.0)
        z_ex = z_ex + s_t[t] * (hx @ Wout)
    bfix = (z_ex - z).mean(axis=0).astype(np.float32)
    return db0, db, bfix


def _build_program(n_steps, s_t, alpha):
    import concourse.bacc as bacc
    import concourse.mybir as mybir
    from concourse import tile
    from contextlib import ExitStack

    F32 = mybir.dt.float32
    F8 = mybir.dt.float8e4
    BF16 = mybir.dt.bfloat16
    AF = mybir.ActivationFunctionType
    ALU = mybir.AluOpType
    DR = mybir.MatmulPerfMode.DoubleRow

    nc = bacc.Bacc("TRN2", target_bir_lowering=False, debug=False, num_devices=N_CORES)

    u_in = nc.declare_dram_parameter("U16", [128, 2, B_CORE], BF16, isOutput=False)
    zs_in = nc.declare_dram_parameter("ZS0", [2, 128, NB], F32, isOutput=False)
    z8_in = nc.declare_dram_parameter("Z80", [2, 128, NB], F8, isOutput=False)
    ident_in = nc.declare_dram_parameter("I16", [128, 128], BF16, isOutput=False)
    w0a_in = nc.declare_dram_parameter("W0A8S", [128, 4, 2, 128], F8, isOutput=False)
    wh8_in = nc.declare_dram_parameter("WH8", [128, 3, 2, 2, 128], F8, isOutput=False)
    wo8_in = nc.declare_dram_parameter("WO8P", [128, 4, 2, 128], F8, isOutput=False)
    btab_in = nc.declare_dram_parameter("BTAB", [n_steps, 128, 8], F32, isOutput=False)
    out_ext = nc.declare_dram_parameter("OUTZ", [2, 128, NB], F32, isOutput=True)

    with tile.TileContext(nc) as tc:
        with ExitStack() as ctx:
            wpool = ctx.enter_context(tc.tile_pool(name="weights", bufs=1))
            zpool = ctx.enter_context(tc.tile_pool(name="zbufs", bufs=1))
            hpool = ctx.enter_context(tc.tile_pool(name="acts", bufs=8))
            bstream = ctx.enter_context(tc.tile_pool(name="bstream", bufs=4))
            ppool = ctx.enter_context(tc.tile_pool(name="psum", bufs=4, space="PSUM"))

            u16 = wpool.tile([128, 2, B_CORE], BF16, tag="u16")
            ident = wpool.tile([128, 128], BF16, tag="ident")
            w0a8 = wpool.tile([128, 4, 2, 128], F8, tag="w0a8")
            wh8 = wpool.tile([128, 3, 2, 2, 128], F8, tag="wh8")
            wo8p = wpool.tile([128, 4, 2, 128], F8, tag="wo8p")
            for j in range(2):
                nc.sync.dma_start(u16[:, j, :], u_in[:, j, :])
            nc.sync.dma_start(ident[:], ident_in[:])
            nc.sync.dma_start(w0a8[:], w0a_in[:])
            nc.sync.dma_start(wh8[:], wh8_in[:])
            nc.sync.dma_start(wo8p[:], wo8_in[:])

            # HAM warmup: the PE idles >3.4us during the initial DMA load,
            # so step 0 would start clock-gated at 1.2GHz. ~48 junk matmuls
            # reading the early-arriving ident tile warm the PE during the
            # DMA phase (results never read; overlaps the u16 transfer).
            wpt = ppool.tile([128, 1024], F32, tag="ps", name="wpt")
            for i in range(48):
                nc.tensor.matmul(wpt[:, :128], ident[:], ident[:],
                                 start=True, stop=True)

            # z state: stacked [128, 512] per half; f32 ping-pong + fp8 single
            zs = [
                [
                    zpool.tile([128, NB], F32, tag=f"zs{p}_{h}", name=f"zs{p}_{h}")
                    for h in range(2)
                ]
                for p in range(2)
            ]
            z8 = [
                zpool.tile([128, NB], F8, tag=f"z8_{h}", name=f"z8_{h}")
                for h in range(2)
            ]
            for h in range(2):
                nc.sync.dma_start(zs[0][h][:], zs_in[h])
                nc.sync.dma_start(z8[h][:], z8_in[h])

            # Whole-tile epilogues, one engine per tile, strictly alternating
            # ACT/DVE in emission order so consecutive tiles drain on
            # different engines (the sync optimizer chains same-tile ops, so
            # column-splitting one tile across engines serializes instead of
            # parallelizing). H-major hidden order puts each half's (j0,j1)
            # pair on different engines, shortening the layer handoff.

            prestarted = {}

            for t in range(n_steps):
                btab = bstream.tile([128, 8], F32, tag="btab", name="btab")
                nc.sync.dma_start(btab[:], btab_in[t])
                zc = zs[t % 2]
                zn = zs[(t + 1) % 2]

                h_cur = {}
                # 2-bank psum tiles (depth-4 pipeline). Per layer, 8 tiles
                # (j, H, bp); engine pattern ADAD/DADA keeps each (H,bp)
                # j-pair on different engines and adjacent tiles alternating.

                def epilogue(pt, h, j, bias_ap, on_act):
                    if on_act:
                        nc.scalar.activation(
                            h[:, j, :], pt[:], AF.Relu, bias=bias_ap
                        )
                    else:
                        nc.vector.tensor_scalar(
                            h[:, j, :], pt[:], bias_ap, 0.0, ALU.add, ALU.max
                        )

                # ---- layer 0: identity-u (bf16) + stacked z8 (fp8) ----
                for H in range(2):
                    for bp in range(2):
                        h_cur[(H, bp)] = hpool.tile(
                            [128, 2, 1024], F8, tag="h", name=f"h0_{H}_{bp}"
                        )
                for li, (H, j, bp) in enumerate(
                    (H, j, bp) for H in range(2) for j in range(2) for bp in range(2)
                ):
                    if (H, j, bp) in prestarted:
                        pt = prestarted.pop((H, j, bp))
                    else:
                        pt = ppool.tile([128, 1024], F32, tag="ps", name="p0")
                        for i in range(2):
                            b = 2 * bp + i
                            col = H * 2048 + b * NB
                            nc.tensor.matmul(
                                pt[:, i * NB : (i + 1) * NB],
                                ident[:],
                                u16[:, j, col : col + NB],
                                start=True,
                                stop=False,
                            )
                    for i in range(2):
                        b = 2 * bp + i
                        nc.tensor.matmul(
                            pt[:, i * NB : (i + 1) * NB],
                            w0a8[:, b, j, :],
                            z8[H][:],
                            start=False,
                            stop=True,
                        )
                    on_act = li in (0, 3, 4, 7)
                    epilogue(pt, h_cur[(H, bp)], j, btab[:, j : j + 1], on_act)

                # ---- hidden layers: fp8 DoubleRow K=256 ----
                for l in range(3):
                    hn = {}
                    for H in range(2):
                        for bp in range(2):
                            hn[(H, bp)] = hpool.tile(
                                [128, 2, 1024], F8, tag="h", name=f"h{l+1}_{H}_{bp}"
                            )
                    for li, (j, H, bp) in enumerate(
                        (j, H, bp) for j in range(2) for H in range(2)
                        for bp in range(2)
                    ):
                        pt = ppool.tile([128, 1024], F32, tag="ps", name="pl")
                        for i in range(2):
                            mm = nc.tensor.matmul(
                                pt[:, i * NB : (i + 1) * NB],
                                wh8[:, l, j, :, :],
                                h_cur[(H, bp)][:, :, i * NB : (i + 1) * NB],
                                start=True,
                                stop=True,
                                perf_mode=DR,
                            )
                            # all 8 MMs of a (l, j) group share stationary
                            # weights and are emission-consecutive on the PE
                            # queue: load them once (fp8 reuse is ISA-legal)
                            if li % 4 != 0 or i != 0:
                                mm.ins.ldweights = False
                        bias_ap = btab[:, 2 + l * 2 + j : 3 + l * 2 + j]
                        # one l2 tile moves DVE->ACT (17A/15D): DVE is the
                        # busiest engine; breaking this one j-pair only
                        # delays the out-phase handoff, which is short.
                        on_act = (li + li // 4) % 2 == 0
                        if l == 2 and li == 6:
                            on_act = True
                        epilogue(pt, hn[(H, bp)], j, bias_ap, on_act)
                    h_cur = hn

                # Pre-start the next step's first two L0 tiles (u-inject
                # half only): they depend only on a free PSUM buf, so one
                # fills the PE gap while po_H0 waits on l3 epilogues and the
                # other fills the gap before po_H1 (po_H1 reuses po_H0's
                # buffer, freed by STT_H0 — no circular wait).
                def prestart(H2, j2, bp2):
                    ptp = ppool.tile([128, 1024], F32, tag="ps", name="pp")
                    for i in range(2):
                        b = 2 * bp2 + i
                        col = H2 * 2048 + b * NB
                        nc.tensor.matmul(
                            ptp[:, i * NB : (i + 1) * NB],
                            ident[:],
                            u16[:, j2, col : col + NB],
                            start=True,
                            stop=False,
                        )
                    prestarted[(H2, j2, bp2)] = ptp

                def emit_out(H):
                    po = ppool.tile([128, 1024], F32, tag="ps", name="po")
                    for q in range(4):
                        nc.tensor.matmul(
                            po[:, :NB],
                            wo8p[:, q, :, :],
                            h_cur[(H, q // 2)][:, :, (q % 2) * NB : (q % 2 + 1) * NB],
                            start=(q == 0),
                            stop=(q == 3),
                            perf_mode=DR,
                        )
                    nc.vector.scalar_tensor_tensor(
                        zn[H][:],
                        po[:, :NB],
                        float(s_t[t]),
                        zc[H][:],
                        ALU.mult,
                        ALU.add,
                    )
                    if t + 1 < n_steps:
                        if H == 0:
                            # DVE, queue-sequential after the STT (no cross-
                            # engine hop) and in DVE's end-of-step bubble;
                            # SBUF->SBUF tensor_scalar hits the 2x_2p path.
                            nc.vector.tensor_scalar(
                                z8[H][:], zn[H][:], float(alpha[t + 1]), None,
                                ALU.mult
                            )
                        else:
                            nc.scalar.activation(
                                z8[H][:], zn[H][:], AF.Copy,
                                scale=float(alpha[t + 1])
                            )

                if t + 1 < n_steps:
                    prestart(0, 0, 0)
                emit_out(0)
                if t + 1 < n_steps:
                    prestart(0, 0, 1)
                emit_out(1)

            zfin = zs[n_steps % 2]
            for h in range(2):
                nc.sync.dma_start(out_ext[h], zfin[h][:])

    nc.compile()
    return nc


def kernel(state, init_noise, W0, b0, Wh, bh, Wout, bout):
    from concourse.bass_utils import run_bass_kernel_spmd

    state = np.ascontiguousarray(np.asarray(state, np.float32))
    init_noise = np.ascontiguousarray(np.asarray(init_noise, np.float32))
    Wh_np = np.asarray(Wh, np.float32)
    bh_np = np.asarray(bh, np.float32)
    Wout_np = np.asarray(Wout, np.float32)

    tb = _host_tables(np.asarray(W0, np.float32), np.asarray(b0, np.float32),
                      np.asarray(bout, np.float32))

    n_steps = int(os.environ.get("DPH_KERNEL_STEPS", N_STEPS))
    db0, db, bfix = _calibrate(state, init_noise, Wh_np, bh_np, Wout_np, tb, n_steps)

    alpha = tb["alpha"].astype(np.float32)
    if _cached.get("nc_steps") != n_steps:
        _cached["nc"] = _build_program(n_steps, tb["s_t"], alpha)
        _cached["nc_steps"] = n_steps
    nc = _cached["nc"]

    # ---- shared tables ----
    eprime_eff = tb["eprime"].astype(np.float32)[:n_steps] + db0  # [n, 256]
    btab = np.empty((n_steps, 128, 8), np.float32)
    for j in range(2):
        btab[:, :, j] = eprime_eff[:, j * 128 : (j + 1) * 128]
    bh_eff = bh_np[None, :, :] + db
    for l in range(3):
        for j in range(2):
            btab[:, :, 2 + l * 2 + j] = bh_eff[:, l, j * 128 : (j + 1) * 128]

    ident = np.eye(128, dtype=ml_dtypes.bfloat16)

    # W0A8S: [p, q, j, m] = q8(W0a[p-32q, j*128+m]) for p in [32q, 32q+32)
    w0a8s = np.zeros((128, 4, 2, 128), ml_dtypes.float8_e4m3)
    w0a8 = np.asarray(tb["W0a"], ml_dtypes.float8_e4m3)  # [32, 256]
    for q in range(4):
        for j in range(2):
            w0a8s[32 * q : 32 * (q + 1), q, j, :] = w0a8[:, j * 128 : (j + 1) * 128]

    # WH8: [p, l, j, c, m] = q8(Wh[l][c*128+p, j*128+m])
    wh8 = np.empty((128, 3, 2, 2, 128), ml_dtypes.float8_e4m3)
    whq = np.asarray(Wh_np, ml_dtypes.float8_e4m3)
    for l in range(3):
        for j in range(2):
            for c in range(2):
                wh8[:, l, j, c, :] = whq[l, c * 128 : (c + 1) * 128,
                                         j * 128 : (j + 1) * 128]

    # WO8P: [p, q, c, m] = q8(Wout[c*128+p, m-32q]) for m in [32q, 32q+32)
    wo8p = np.zeros((128, 4, 2, 128), ml_dtypes.float8_e4m3)
    woq = np.asarray(Wout_np, ml_dtypes.float8_e4m3)  # [256, 32]
    for q in range(4):
        for c in range(2):
            wo8p[:, q, c, 32 * q : 32 * (q + 1)] = woq[c * 128 : (c + 1) * 128, :]

    # u = state @ W0s on host (f32), bf16 on device
    u_full = state @ tb["W0s"]  # [BATCH, 256]

    in_maps = []
    for core in range(N_CORES):
        rows = slice(core * B_CORE, (core + 1) * B_CORE)
        u_core = np.ascontiguousarray(u_full[rows].T)  # [256, 4096]
        u16 = u_core.reshape(2, 128, B_CORE).transpose(1, 0, 2)  # [128, 2, 4096]
        # stacked z0: zs[H][32q+r, c] = noise[core*4096 + (4H+q)*512 + c, r]
        nz = init_noise[rows].reshape(8, NB, ACTION_DIM)  # [b, c, r]
        zs0 = np.zeros((2, 128, NB), np.float32)
        for H in range(2):
            for q in range(4):
                zs0[H, 32 * q : 32 * (q + 1), :] = nz[4 * H + q].T
        z80 = np.asarray(alpha[0] * zs0, ml_dtypes.float8_e4m3)
        in_maps.append(
            {
                "U16": np.ascontiguousarray(u16.astype(ml_dtypes.bfloat16)),
                "ZS0": zs0,
                "Z80": z80,
                "I16": ident,
                "W0A8S": w0a8s,
                "WH8": wh8,
                "WO8P": wo8p,
                "BTAB": btab,
            }
        )

    _cached["in_maps"] = in_maps
    res = run_bass_kernel_spmd(nc, in_maps, core_ids=list(range(N_CORES)))
    _cached["last_results"] = res

    g50 = np.float32(tb["g_final"])
    beta50 = tb["beta_final"].astype(np.float32)
    out = np.empty((BATCH, ACTION_DIM), np.float32)
    for core in range(N_CORES):
        oz = res.results[core]["OUTZ"]  # [2, 128, 512]
        zdec = np.empty((8, NB, ACTION_DIM), np.float32)  # [b, c, r]
        for H in range(2):
            for q in range(4):
                zdec[4 * H + q] = oz[H, 32 * q : 32 * (q + 1), :].T
        rows = slice(core * B_CORE, (core + 1) * B_CORE)
        out[rows] = g50 * (zdec.reshape(B_CORE, ACTION_DIM) + bfix) + beta50
    return out


if __name__ == "__main__":
    _c = np.load("/root/problem/ref_cache.npz")
    inputs = {k: _c[k] for k in _c.files if k != "expected"}
    got = kernel(**inputs)
    exp = _c["expected"]
    d = np.linalg.norm(got - exp) / np.linalg.norm(exp)
    print(f"L2 relative error: {d:.4e}")
